# revision 62
# baseline (speedup 1.0000x reference)
"""GCN embedder kernel for TRN2, 8-core SPMD (v7: 256-wide dst windows,
pure-one-hot scatter matmuls, dinv factoring, unified 3-layer edge pass,
split/parity-double-buffered table AllGathers).

Design
------
* Nodes sharded contiguously across C=8 cores (NC=12500 each). Edges are
  owned by the dst core. Self-loops are NOT materialized as edges.
* Normalization factored: norm(s,d) = dinv[s]*dinv[d]. dinv[src] is
  pre-multiplied into the gather table rows (T'[v] = dinv[v] * (h@W)[v]),
  dinv[dst] is applied at PSUM flush (tensor_tensor with a row-replicated
  dinvR). The self-loop term dinv[d]^2*(h@W)[d] = dinv[d]*T'[d] is
  injected by an identity matmul of the local feature-major T' into PSUM,
  opening each quad's accumulation group.
* Edge pass (identical structure for all 3 layers): edges sorted by
  (window=256 dsts, chunk, dst). Per (window, chunk) group, ALL tiles'
  one-hot rows B[e, t, drel] = is_equal(iota, drel) are built in ONE DVE
  tensor_tensor using a stride-0 broadcast AP over the bf16 drel metadata
  (per-instruction DVE fixed cost ~600ns makes per-tile builds ruinous).
  One PE matmul per 128-edge tile (lhsT=messages, rhs=B [128,256])
  accumulates feature-major into the 512-col quad PSUM bank.
* Gathers: SWDGE dma_gather, ONE call per (window, chunk) group
  (~196/layer x ~1.1K indices; 994ns fixed GpSimd cost per call makes
  small calls ruinous, ring-drain stalls make much bigger ones slow).
  Table rows bf16 256B; int16 indices, chunk-relative. The SAME
  index/meta streams serve all 3 layers (layer 1's table is per-node:
  T1n[v] = dinv[v]*T1[x[v]], vocab-gathered on chip).
* Tables are split in half (A = shard rows [0:6250), B = rest) and
  double-buffered by layer parity. The A-half AllGather fires mid-edge-
  pass (as soon as quad 12's table phase wrote its rows) and overlaps
  the producing layer's tail; chunks are defined as (half, core-group)
  so each chunk is one contiguous 25000-row region of fullA/fullB.
  Gather issue order puts A-half chunks first so they proceed while the
  B AllGather is still in flight.
* Table phase (interleaved into the edge pass of the previous layer at
  quad granularity): hwT = W^T @ hT (PE), T'T = hwT * dinvR (DVE),
  transpose blocks to rows (PE), DMA to the split DRAM shards.
* Pooling: transpose h3 blocks to node-major (PE), then is_equal
  (batchrel) one-hot matmuls (lhsT=Bpool half, rhs=h3 block)
  accumulating [grel<=256, H] in 2 PSUM banks; flush rows scatter by
  graph id (indirect DMA), AllReduce, scale by 1/cnt.

All structure (tile counts, call sizes) is maxed across cores so the
single SPMD program fits every core; pad slots have drel=-1 (zero
one-hot column) and index 0 (valid row). NOTE: single_packet=True
works only for gather calls up to ~640 indices (hangs beyond) - the
per-group calls here are larger, so it stays False.
"""

import math
from contextlib import ExitStack
from dataclasses import dataclass, field

import numpy as np

import concourse.mybir as mybir
import concourse.tile as tile
from concourse import bacc, bass
from concourse.bass import AP, IndirectOffsetOnAxis, ds
from concourse.masks import make_identity

F32 = mybir.dt.float32
BF16 = mybir.dt.bfloat16
I16 = mybir.dt.int16
I32 = mybir.dt.int32
AF = mybir.ActivationFunctionType
OP = mybir.AluOpType

P = 128  # partitions / hidden size / vocab

DEBUG_STAGE = 0  # 0=off; 1..3 = dump h after that layer


@dataclass
class Cfg:
    N: int = 100000
    E: int = 1600000
    H: int = 128
    V: int = 128
    L: int = 3
    G: int = 1024
    C: int = 8          # cores
    CH: int = 4         # gather-table chunks (int16 index limit)
    WIN: int = 256      # dst window width
    NQ: int = 4         # SWDGE queues
    SCALAR_FRAC_NUM: int = 1   # of every DEN tiles, NUM one-hots on Scalar
    SCALAR_FRAC_DEN: int = 7

    @property
    def NC(self):
        assert self.N % self.C == 0
        return self.N // self.C

    @property
    def CHN(self):
        assert self.N % self.CH == 0
        return self.N // self.CH

    @property
    def W(self):  # dst windows per core
        return math.ceil(self.NC / self.WIN)

    @property
    def WPQ(self):  # windows per 512-col PSUM quad
        return 512 // self.WIN

    @property
    def Q(self):  # 512-wide quads (ranges) per core
        return math.ceil(self.W / self.WPQ)

    @property
    def NB(self):  # 128-node blocks per core (boot/pool granularity)
        return math.ceil(self.NC / 128)

    @property
    def NCP(self):
        return self.W * self.WIN

    @property
    def GSPAN(self):
        return 256


@dataclass
class Structure:
    # t_cw[c][w]: edge tiles for (chunk c, window w), maxed across cores
    t_cw: list = field(default_factory=list)

    @property
    def T(self):
        return sum(sum(r) for r in self.t_cw)


def preprocess(x, edge_index, batch, emb_table, Ws, bs, cfg: Cfg):
    """Host-side (index-only) preprocessing."""
    N, E, C, CH = cfg.N, cfg.E, cfg.C, cfg.CH
    NC, CHN, W, Q, WIN = cfg.NC, cfg.CHN, cfg.W, cfg.Q, cfg.WIN

    x = np.asarray(x).astype(np.int64)
    edge_index = np.asarray(edge_index).astype(np.int64)
    batch = np.asarray(batch).astype(np.int64)

    src, dst = edge_index[0], edge_index[1]
    deg = (np.bincount(dst, minlength=N) + 1).astype(np.float32)  # + self
    dinv = (1.0 / np.sqrt(deg)).astype(np.float32)

    owner = dst // NC
    NC2 = NC // 2
    per_core = []
    for c in range(C):
        m = owner == c
        s_c = src[m]
        d_c = dst[m] - c * NC
        w_c = d_c // WIN
        # chunk = (half of the owner's shard) * 2 + (owner core >= 4);
        # matches the split-AllGather table layout fullA/fullB[(k%4)*NC2+...]
        k = s_c // NC
        loc = s_c % NC
        half = loc // NC2
        ck = half * 2 + (k >= C // 2)
        srel = (k % (C // 2)) * NC2 + (loc % NC2)
        o = np.lexsort((d_c, ck, w_c))  # window-major, then chunk, then dst
        per_core.append(dict(s=srel[o], drel=(d_c[o] % WIN),
                             grp=(w_c[o] * CH + ck[o])))

    # tiles per (window, chunk) group, maxed across cores
    NG = W * CH
    t_g = np.zeros(NG, dtype=np.int64)
    for c in range(C):
        cnt = np.bincount(per_core[c]["grp"], minlength=NG)
        t_g = np.maximum(t_g, -(-cnt // P))
    t_g = np.maximum(t_g, 1)
    st = Structure(t_cw=[list(map(int, t_g))])
    T = int(t_g.sum())
    gt0 = np.concatenate([[0], np.cumsum(t_g)[:-1]])  # tile offset per group

    def wrap(arr):
        # [n] int16 -> [128, n//16] wrapped in 16 partitions, tiled x8
        wr = arr.reshape(-1, 16).T
        return np.tile(wr, (8, 1))

    in_maps = []
    for c in range(C):
        pc = per_core[c]
        cnt = np.bincount(pc["grp"], minlength=NG)
        starts = np.concatenate([[0], np.cumsum(cnt)[:-1]])

        # meta + idx stream, both in (window, chunk) consumption order;
        # pads: drel=-1 (zero one-hot col), idx=-1 (trailing - skipped by
        # the gather via the per-core real-count register)
        meta = np.full((P, T), -1.0, dtype=np.float32)
        idxs = np.zeros(T * P, dtype=np.int16)
        for g in range(NG):
            n = int(cnt[g])
            sl = slice(int(starts[g]), int(starts[g]) + n)
            ii = np.arange(n)
            t0 = int(gt0[g])
            meta[ii % P, t0 + ii // P] = pc["drel"][sl]
            idxs[t0 * P + ii] = pc["s"][sl].astype(np.int16)
        gidx = np.zeros((P, T * 8), dtype=np.int16)
        for g in range(NG):
            t0, nt = int(gt0[g]), int(t_g[g])
            gidx[:, t0 * 8:(t0 + nt) * 8] = wrap(idxs[t0 * P:(t0 + nt) * P])
        cnts = np.broadcast_to(cnt.astype(np.int32)[None, :],
                               (P, NG)).copy()

        # vocab gather indices for the layer-1 table (node-major, pad->0)
        NB = cfg.NB
        nodes = np.arange(cfg.NCP) + c * NC
        valid = nodes < (c + 1) * NC
        xl = np.where(valid, x[np.minimum(nodes, N - 1)], 0)
        xidx = wrap(xl.astype(np.int16))  # [128, NB*8]

        dloc = np.where(valid, dinv[np.minimum(nodes, N - 1)], 0.0)
        dinv_pm = dloc.reshape(NB, P).T.copy().astype(np.float32)  # [128, NB]
        dinvR = np.broadcast_to(dloc[None, :], (P, cfg.NCP)).astype(np.float32)

        # pooling metadata (as baseline)
        bvals = np.where(valid, batch[np.minimum(nodes, N - 1)], -1)
        gmin = int(batch[c * NC])
        gmax = int(batch[min((c + 1) * NC, N) - 1])
        assert gmax - gmin < cfg.GSPAN, (c, gmin, gmax)
        brel = np.where(valid, bvals - gmin, -1).astype(np.float32)
        pool_meta = brel.reshape(NB, P).T.copy()  # [128, NB]
        gid_rows = gmin + np.arange(cfg.GSPAN)
        gid_rows = np.where(gid_rows < cfg.G, gid_rows,
                            cfg.G + np.arange(cfg.GSPAN) % 256).astype(np.int32)
        gid_cols = gid_rows.reshape(2, P).T.copy()  # [128, 2]

        cnts = np.bincount(batch, minlength=cfg.G).astype(np.float32)
        recip = 1.0 / np.maximum(cnts, 1.0)
        recip_pm = recip.reshape(cfg.G // P, P).T.copy()

        in_maps.append({
            "meta": meta, "gidx": gidx,
            "xidx": np.ascontiguousarray(xidx),
            "dinv_pm": dinv_pm, "dinvR": np.ascontiguousarray(dinvR),
            "pool_meta": pool_meta, "gid_cols": gid_cols,
            "recip_pm": recip_pm,
            "emb": np.asarray(emb_table, dtype=np.float32),
            "Ws": np.asarray(Ws, dtype=np.float32),
            "bs": np.asarray(bs, dtype=np.float32),
        })

    # per-window tiles and per-quad totals
    WPQ = cfg.WPQ
    t_w = t_g.reshape(W, CH).sum(axis=1)
    nt_q = [int(t_w[WPQ * q:min(WPQ * q + WPQ, W)].sum()) for q in range(Q)]
    st.t_g = [int(v) for v in t_g]
    st.t_w = [int(v) for v in t_w]
    st.nt_q = nt_q
    st.NTQ = max(nt_q)
    st.NTG = int(t_g.max())
    st.NG = NG
    return st, in_maps


# --------------------------------------------------------------------------
# device program
# --------------------------------------------------------------------------

def build_nc(cfg: Cfg, st: Structure):
    N, H, C, CH, W, Q = cfg.N, cfg.H, cfg.C, cfg.CH, cfg.W, cfg.Q
    NC, CHN, NCP, WIN = cfg.NC, cfg.CHN, cfg.NCP, cfg.WIN
    T = st.T
    NTQ = st.NTQ
    NB = cfg.NB
    WPQ = cfg.WPQ
    GS = cfg.GSPAN
    GW = cfg.G // P
    NQ = cfg.NQ

    nc = bacc.Bacc(None, num_devices=C, num_swdge_queues=NQ)
    cores = list(range(C))

    # ---- external I/O ----
    meta_d = nc.declare_dram_parameter("meta", [P, T], F32, isOutput=False)
    gidx_d = nc.declare_dram_parameter("gidx", [P, T * 8], I16, isOutput=False)
    xidx_d = nc.declare_dram_parameter("xidx", [P, NB * 8], I16, isOutput=False)
    dinv_pm_d = nc.declare_dram_parameter("dinv_pm", [P, NB], F32, isOutput=False)
    dinvR_d = nc.declare_dram_parameter("dinvR", [P, NCP], F32, isOutput=False)
    pool_meta = nc.declare_dram_parameter("pool_meta", [P, NB], F32, isOutput=False)
    gid_cols = nc.declare_dram_parameter("gid_cols", [P, 2], I32, isOutput=False)
    recip_pm = nc.declare_dram_parameter("recip_pm", [P, GW], F32, isOutput=False)
    emb_d = nc.declare_dram_parameter("emb", [P, H], F32, isOutput=False)
    Ws_d = nc.declare_dram_parameter("Ws", [cfg.L, H, H], F32, isOutput=False)
    bs_d = nc.declare_dram_parameter("bs", [cfg.L, H], F32, isOutput=False)
    out_d = nc.declare_dram_parameter("out", [cfg.G, H], F32, isOutput=True)

    # ---- internal DRAM ----
    # gather tables double-buffered by layer parity; each split in half
    # (A = local rows [0:NC2), B = [NC2:NC)) so the A AllGather can fire
    # mid-edge-pass and overlap the producing layer's tail
    NC2 = NC // 2
    t1_dram = nc.dram_tensor("t1_tab", [cfg.V, H], BF16)
    shardA = [nc.dram_tensor(f"shardA{i}", [NC2, H], BF16) for i in range(2)]
    shardB = [nc.dram_tensor(f"shardB{i}", [NC2, H], BF16) for i in range(2)]
    fullA = [nc.dram_tensor(f"fullA{i}", [C * NC2, H], BF16,
                            addr_space="Shared") for i in range(2)]
    fullB = [nc.dram_tensor(f"fullB{i}", [C * NC2, H], BF16,
                            addr_space="Shared") for i in range(2)]
    pooled_nm = nc.dram_tensor("pooled_nm", [cfg.G + GS, H], F32)
    pooled_sum = nc.dram_tensor("pooled_sum", [cfg.G + GS, H], F32,
                                addr_space="Shared")

    def shard_rows_dma(par, t, tile_, islot, nq):
        # write node-major block t (rows t*128..t*128+nq) into the split
        # shard tensors of parity `par`; tile_ is [P, ..., H], islot selects
        # the middle index (None for 2D tiles)
        def rows(a, b):
            return tile_[a:b, :] if islot is None else tile_[a:b, islot, :]
        lo, hi = t * P, t * P + nq
        if hi <= NC2:
            nc.sync.dma_start(out=shardA[par][lo:hi, :], in_=rows(0, nq))
        elif lo >= NC2:
            nc.sync.dma_start(out=shardB[par][lo - NC2:hi - NC2, :],
                              in_=rows(0, nq))
        else:
            na = NC2 - lo
            nc.sync.dma_start(out=shardA[par][lo:NC2, :], in_=rows(0, na))
            nc.sync.dma_start(out=shardB[par][0:hi - NC2, :],
                              in_=rows(na, nq))

    def allgather_half(par, half):
        sh = shardA[par] if half == 0 else shardB[par]
        fu = fullA[par] if half == 0 else fullB[par]
        nc.gpsimd.collective_compute(
            "AllGather", OP.bypass, replica_groups=[cores],
            ins=[sh[:, :]], outs=[fu[:, :]])

    from concourse.tile import add_dep_helper
    pd = {"i": 0, "last": None}

    def chain_pool_dma(inst, chain=True):
        if chain and pd["last"] is not None:
            add_dep_helper(inst.ins, pd["last"].ins, sync=False,
                           reason="pool-dma queue/lane parity order")
        pd["last"] = inst
        pd["i"] += 1

    with tile.TileContext(nc) as tc, ExitStack() as ctx:
        const = ctx.enter_context(tc.tile_pool(name="const", bufs=1))
        hpool = ctx.enter_context(tc.tile_pool(name="hbuf", bufs=1))

        ident = const.tile([P, P], F32)
        make_identity(nc, ident[:])
        ident_bf = const.tile([P, P], BF16)
        make_identity(nc, ident_bf[:])
        iota_i = const.tile([P, 512], I32)
        nc.gpsimd.iota(iota_i[:], pattern=[[1, 512]], base=0,
                       channel_multiplier=0)
        iota_pool = const.tile([P, GS], BF16)
        nc.vector.tensor_copy(out=iota_pool[:], in_=iota_i[:, :GS])
        # repeating 0..WIN-1 pattern, one block per tile of a window group
        NTG2 = 2 * st.NTG
        iotaB_i = const.tile([P, NTG2, WIN], I32)
        nc.gpsimd.iota(iotaB_i[:], pattern=[[0, NTG2], [1, WIN]], base=0,
                       channel_multiplier=0)
        iotaB = const.tile([P, NTG2, WIN], BF16)
        nc.vector.tensor_copy(out=iotaB[:], in_=iotaB_i[:])

        b_cols = const.tile([P, cfg.L], F32)
        for l in range(cfg.L):
            nc.sync.dma_start(out=b_cols[:, l:l + 1], in_=bs_d[l, :, None])
        w_bf = const.tile([P, cfg.L * H], BF16, tag="w_bf")
        with tc.tile_pool(name="wload", bufs=2) as wl:
            for l in range(cfg.L):
                wt = wl.tile([P, H], F32, tag="wt")
                nc.sync.dma_start(out=wt[:], in_=Ws_d[l])
                nc.vector.tensor_copy(out=w_bf[:, l * H:(l + 1) * H], in_=wt[:])
        dinv_pm = const.tile([P, NB], F32)
        nc.sync.dma_start(out=dinv_pm[:], in_=dinv_pm_d[:, :])

        # resident meta (bf16: drel in 0..127 and -1 are exact) + dinvR (bf16)
        meta_bf = const.tile([P, T], BF16, tag="meta_bf")
        with tc.tile_pool(name="mld", bufs=2) as mld:
            MC = 1024
            for s0 in range(0, T, MC):
                nn = min(MC, T - s0)
                mt_ = mld.tile([P, MC], F32, tag="m")
                nc.sync.dma_start(out=mt_[:, :nn], in_=meta_d[:, s0:s0 + nn])
                nc.vector.tensor_copy(out=meta_bf[:, s0:s0 + nn],
                                      in_=mt_[:, :nn])
        dinvR = const.tile([P, NCP], BF16, tag="dinvR")
        with tc.tile_pool(name="dld", bufs=2) as dld:
            for s0 in range(0, NCP, 512):
                nn = min(512, NCP - s0)
                dt_ = dld.tile([P, 512], F32, tag="d")
                nc.sync.dma_start(out=dt_[:, :nn], in_=dinvR_d[:, s0:s0 + nn])
                nc.vector.tensor_copy(out=dinvR[:, s0:s0 + nn],
                                      in_=dt_[:, :nn])

        hT_a = hpool.tile([P, NCP], BF16)     # feature-major h (layers 1,2)
        hT_b = hpool.tile([P, NCP], BF16)
        TpT = hpool.tile([P, NCP], BF16)      # feature-major local T'
        h3nm = hT_a                           # layer-3 out (node-major) aliases
        #                                       layer-1 h (dead by then)

        # ---------------- layer-1 table: T1n[v] = dinv[v]*T1[x[v]] ----------
        with nc.named_scope("boot"), \
             tc.tile_pool(name="pro", bufs=2) as pro, \
             tc.tile_pool(name="pro_ps", bufs=2, space="PSUM") as pro_ps, \
             tc.tile_pool(name="bootg", bufs=2) as bootg, \
             tc.tile_pool(name="bootix", bufs=1) as bootix:
            emb_sb = pro.tile([P, H], F32, tag="emb")
            nc.sync.dma_start(out=emb_sb[:], in_=emb_d[:, :])
            w1_sb = pro.tile([P, H], F32, tag="w1")
            nc.sync.dma_start(out=w1_sb[:], in_=Ws_d[0])
            embT_ps = pro_ps.tile([P, P], F32)
            nc.tensor.transpose(out=embT_ps[:], in_=emb_sb[:], identity=ident[:])
            embT = pro.tile([P, P], F32, tag="embT")
            nc.vector.tensor_copy(out=embT[:], in_=embT_ps[:])
            t1t_ps = pro_ps.tile([P, P], F32)
            nc.tensor.matmul(out=t1t_ps[:], lhsT=w1_sb[:], rhs=embT[:],
                             start=True, stop=True)
            t1t = pro.tile([P, P], F32, tag="t1t")
            nc.vector.tensor_copy(out=t1t[:], in_=t1t_ps[:])
            t1nm_ps = pro_ps.tile([P, P], F32)
            nc.tensor.transpose(out=t1nm_ps[:], in_=t1t[:], identity=ident[:])
            t1nm = pro.tile([P, P], BF16, tag="t1nm")
            nc.vector.tensor_copy(out=t1nm[:], in_=t1nm_ps[:])
            nc.sync.dma_start(out=t1_dram[:, :], in_=t1nm[:])

            # vocab gather (node-major), scale by dinv, rows -> tab_shard,
            # transpose -> TpT
            xix = bootix.tile([P, NB * 8], I16, tag="xix")
            nc.sync.dma_start(out=xix[:], in_=xidx_d[:, :])
            BG = 14  # tiles per vocab-gather call
            for t0 in range(0, NB, BG):
                nt = min(BG, NB - t0)
                g = bootg.tile([P, BG, H], BF16, tag="vg")
                nsub = min(NQ, nt)
                per = -(-nt // nsub)
                s0 = 0
                while s0 < nt:
                    sn = min(per, nt - s0)
                    gi = nc.gpsimd.dma_gather(
                        out_ap=g[:, s0:s0 + sn, :], in_ap=t1_dram[:, :],
                        idxs_ap=xix[:, (t0 + s0) * 8:(t0 + s0 + sn) * 8],
                        num_idxs=sn * P, num_idxs_reg=sn * P,
                        elem_size=H, single_packet=True,
                        queue_num=pd["i"] % NQ)
                    chain_pool_dma(gi)
                    s0 += sn
                # scale whole call's tiles by per-node dinv in one DVE op
                mnb = bootg.tile([P, BG, H], BF16, tag="mn")
                dbase = dinv_pm[:, t0:t0 + nt]
                dbc = bass.AP(dbase.tensor, dbase.offset,
                              list(dbase.ap) + [[0, H]])
                nc.vector.tensor_tensor(out=mnb[:, :nt, :], in0=g[:, :nt, :],
                                        in1=dbc, op=OP.mult)
                for i in range(nt):
                    t = t0 + i
                    nq = min(P, NC - t * P)
                    if nq <= 0:
                        break
                    shard_rows_dma(0, t, mnb, i, nq)
                    tp_ps = pro_ps.tile([P, P], BF16, tag="tp")
                    nc.tensor.transpose(out=tp_ps[:], in_=mnb[:, i, :],
                                        identity=ident_bf[:])
                    nc.scalar.activation(out=TpT[:, t * P:(t + 1) * P],
                                         in_=tp_ps[:], func=AF.Copy)
                if t0 <= (NC2 // P) < t0 + BG:
                    allgather_half(0, 0)  # A rows complete
            allgather_half(0, 1)

        # ---------------- unified edge pass ----------------
        t_g = st.t_g
        t_w = st.t_w
        nt_q = st.nt_q
        # tile offset of group (w, c) and quad q in the window-major stream
        gt0 = [0] * (W * CH)
        for g in range(1, W * CH):
            gt0[g] = gt0[g - 1] + t_g[g - 1]
        qt0 = [0] * Q
        for q in range(1, Q):
            qt0[q] = qt0[q - 1] + nt_q[q - 1]

        QW = max(nt_q) * 8  # gidx cols per quad (upper bound)

        def edge_pass(layer, h_out):
            lname = f"layer{layer + 1}"
            par = layer % 2
            chunk_aps = [fullA[par][0:CHN, :], fullA[par][CHN:2 * CHN, :],
                         fullB[par][0:CHN, :], fullB[par][CHN:2 * CHN, :]]
            with nc.named_scope(lname), \
                 tc.tile_pool(name=f"ix{layer}", bufs=2) as ixp, \
                 tc.tile_pool(name=f"gb{layer}", bufs=2) as gb, \
                 tc.tile_pool(name=f"bq{layer}", bufs=3) as bq, \
                 tc.tile_pool(name=f"fl{layer}", bufs=3) as fl, \
                 tc.tile_pool(name=f"eps{layer}", bufs=3, space="PSUM") as eps, \
                 tc.tile_pool(name=f"tps{layer}", bufs=2, space="PSUM") as tps, \
                 tc.tile_pool(name=f"tps2{layer}", bufs=2, space="PSUM") as tps2, \
                 tc.tile_pool(name=f"tbl{layer}", bufs=3) as tbl:

                gbuf = {}

                def issue_range(q):
                    # per-(window, chunk) gather calls on rotating queues
                    nt = nt_q[q]
                    gx = ixp.tile([P, QW], I16, tag="gx")
                    nc.sync.dma_start(
                        out=gx[:, :nt * 8],
                        in_=gidx_d[:, qt0[q] * 8:(qt0[q] + nt) * 8])
                    g = gb.tile([P, NTQ, H], BF16, tag="g")
                    # A-half chunks (0,1) first: they only depend on the
                    # early AllGather, so they proceed while AG-B is in
                    # flight instead of blocking the engine behind it
                    for cpair in ((0, 1), (2, 3)):
                        for b in range(WPQ):
                            w = q * WPQ + b
                            if w >= W:
                                break
                            for c in cpair:
                                gi_ = w * CH + c
                                tg = t_g[gi_]
                                so = gt0[gi_] - qt0[q]
                                gcall = nc.gpsimd.dma_gather(
                                    out_ap=g[:, so:so + tg, :],
                                    in_ap=chunk_aps[c],
                                    idxs_ap=gx[:, so * 8:(so + tg) * 8],
                                    num_idxs=tg * P, num_idxs_reg=tg * P,
                                    elem_size=H, single_packet=False,
                                    queue_num=pd["i"] % NQ)
                                chain_pool_dma(gcall, chain=False)
                    gbuf[q] = g

                def build_B(mcol, ntg):
                    # one-hot rows for a (window, chunk-pair) in ONE DVE op:
                    # B[p, t, j] = (iota[j] == drel[p, mcol+t])
                    Bw = bq.tile([P, NTG2, WIN], BF16, tag="Bw")
                    base = meta_bf[:, mcol:mcol + ntg]
                    mb = bass.AP(base.tensor, base.offset,
                                 list(base.ap) + [[0, WIN]])
                    nc.vector.tensor_tensor(
                        out=Bw[:, :ntg, :], in0=iotaB[:, :ntg, :],
                        in1=mb, op=OP.is_equal)
                    return Bw

                def table_phase_quad(q, nxt_layer):
                    # hw for quad q of h_out -> T' rows + TpT (for next layer)
                    ncol = min(512, NCP - q * 512)
                    hw_ps = tps.tile([P, 512], F32, tag="hw")
                    nc.tensor.matmul(
                        out=hw_ps[:, :ncol],
                        lhsT=w_bf[:, nxt_layer * H:(nxt_layer + 1) * H],
                        rhs=h_out[:, q * 512:q * 512 + ncol],
                        start=True, stop=True)
                    nc.vector.tensor_tensor(
                        out=TpT[:, q * 512:q * 512 + ncol],
                        in0=hw_ps[:, :ncol],
                        in1=dinvR[:, q * 512:q * 512 + ncol], op=OP.mult)
                    for b in range(4):
                        t = q * 4 + b
                        if t >= NB:
                            break
                        nq = min(P, NC - t * P)
                        if nq <= 0:
                            break
                        tp_ps = tps2.tile([P, P], BF16, tag="tr")
                        nc.tensor.transpose(
                            out=tp_ps[:], in_=TpT[:, t * P:(t + 1) * P],
                            identity=ident_bf[:])
                        stg = tbl.tile([P, P], BF16, tag="stg")
                        nc.scalar.activation(out=stg[:], in_=tp_ps[:],
                                             func=AF.Copy)
                        shard_rows_dma(1 - par, t, stg, None, nq)

                issue_range(0)
                for q in range(Q):
                    if q + 1 < Q:
                        issue_range(q + 1)
                    ncol = min(512, NCP - q * 512)
                    qpsum = eps.tile([P, 512], F32, tag="qp")
                    # self-loop injection (opens the accumulation group)
                    nc.tensor.matmul(
                        out=qpsum[:, :ncol], lhsT=ident_bf[:],
                        rhs=TpT[:, q * 512:q * 512 + ncol],
                        start=True, stop=False)
                    # edge matmuls (B built per chunk-PAIR to amortize
                    # the ~600ns DVE per-instruction fixed cost)
                    g = gbuf[q]
                    done = 0
                    for b in range(WPQ):
                        w = q * WPQ + b
                        if w >= W:
                            break
                        reg = qpsum[:, b * WIN:(b + 1) * WIN]
                        for c0 in (0, 2):
                            g0, g1 = w * CH + c0, w * CH + c0 + 1
                            tg0, tg1 = t_g[g0], t_g[g1]
                            so = gt0[g0] - qt0[q]
                            Bw = build_B(gt0[g0], tg0 + tg1)
                            for i in range(tg0 + tg1):
                                done += 1
                                nc.tensor.matmul(
                                    out=reg, lhsT=g[:, so + i, :],
                                    rhs=Bw[:, i, :], start=False,
                                    stop=done == nt_q[q])
                    # flush
                    tmp = fl.tile([P, 512], BF16, tag="tmp")
                    nc.vector.tensor_tensor(
                        out=tmp[:, :ncol], in0=qpsum[:, :ncol],
                        in1=dinvR[:, q * 512:q * 512 + ncol], op=OP.mult)
                    func = AF.Relu if layer < cfg.L - 1 else AF.Identity
                    nc.scalar.activation(
                        out=h_out[:, q * 512:q * 512 + ncol],
                        in_=tmp[:, :ncol], func=func,
                        bias=b_cols[:, layer:layer + 1], scale=1.0)
                    # interleaved next-phase work; the A-half AllGather for
                    # the next layer's table fires as soon as its rows are
                    # written (quad NC2//512) and overlaps this layer's tail
                    if layer < cfg.L - 1 and DEBUG_STAGE == 0:
                        table_phase_quad(q, layer + 1)
                        if q == (NC2 - 1) // 512:
                            allgather_half(1 - par, 0)
                if layer < cfg.L - 1:
                    if DEBUG_STAGE != 0:
                        for q in range(Q):
                            table_phase_quad(q, layer + 1)
                        allgather_half(1 - par, 0)
                    allgather_half(1 - par, 1)

        def dump_h(src_tile):
            dbg_d = nc.declare_dram_parameter("dbg", [P, NCP], F32,
                                              isOutput=True)
            with tc.tile_pool(name="dbg", bufs=2) as dbp:
                CWD = 512
                for s0 in range(0, NCP, CWD):
                    nn = min(CWD, NCP - s0)
                    dt_ = dbp.tile([P, CWD], F32, tag="d")
                    nc.vector.tensor_copy(out=dt_[:, :nn],
                                          in_=src_tile[:, s0:s0 + nn])
                    nc.sync.dma_start(out=dbg_d[:, s0:s0 + nn],
                                      in_=dt_[:, :nn])

        with tc.tile_pool(name="zr", bufs=1) as zr:
            zt = zr.tile([P, P], F32, tag="zt")
            nc.vector.memset(zt[:], 0.0)
            for r0 in range(0, cfg.G + GS, P):
                nc.sync.dma_start(out=pooled_nm[r0:r0 + P, :], in_=zt[:])

        edge_pass(0, hT_a)
        if DEBUG_STAGE == 1:
            dump_h(hT_a)
        edge_pass(1, hT_b)
        if DEBUG_STAGE == 2:
            dump_h(hT_b)
        edge_pass(2, h3nm)
        if DEBUG_STAGE == 3:
            dump_h(h3nm)

        # ---------------- pooling ----------------
        with nc.named_scope("pool"), \
             tc.tile_pool(name="po", bufs=3) as po, \
             tc.tile_pool(name="po_ps", bufs=2, space="PSUM") as po_ps, \
             tc.tile_pool(name="po_acc", bufs=2, space="PSUM") as po_acc:
            pm = po.tile([P, NB], F32, tag="pm")
            nc.sync.dma_start(out=pm[:], in_=pool_meta[:, :])
            gcols = po.tile([P, 2], I32, tag="gcols")
            nc.sync.dma_start(out=gcols[:], in_=gid_cols[:, :])
            recip_sb = po.tile([P, GW], F32, tag="recip")
            nc.sync.dma_start(out=recip_sb[:], in_=recip_pm[:, :])

            acc0 = po_acc.tile([P, P], F32)
            acc1 = po_acc.tile([P, P], F32)
            for t in range(NB):
                # h3 is feature-major; transpose the block to node-major
                pt_ps = po_ps.tile([P, P], BF16, tag="ptr")
                nc.tensor.transpose(out=pt_ps[:],
                                    in_=h3nm[:, t * P:(t + 1) * P],
                                    identity=ident_bf[:])
                blk = po.tile([P, P], BF16, tag="blk")
                nc.scalar.activation(out=blk[:], in_=pt_ps[:], func=AF.Copy)
                Bp = po.tile([P, GS], BF16, tag="Bp")
                nc.vector.tensor_scalar(
                    out=Bp[:], in0=iota_pool[:],
                    scalar1=pm[:, t:t + 1], scalar2=None,
                    op0=OP.is_equal)
                nc.tensor.matmul(out=acc0[:], lhsT=Bp[:, :P], rhs=blk[:],
                                 start=(t == 0), stop=(t == NB - 1))
                nc.tensor.matmul(out=acc1[:], lhsT=Bp[:, P:], rhs=blk[:],
                                 start=(t == 0), stop=(t == NB - 1))

            def dummy_gather():
                dz = po.tile([P, 1, P], BF16, tag="dz")
                zi = po.tile([P, 8], I16, tag="zi")
                nc.vector.memset(zi[:], 0)
                gi = nc.gpsimd.dma_gather(
                    out_ap=dz[:], in_ap=t1_dram[:, :], idxs_ap=zi[:],
                    num_idxs=P, num_idxs_reg=P, elem_size=H,
                    single_packet=True, queue_num=pd["i"] % NQ)
                chain_pool_dma(gi)

            for half, acc in ((0, acc0), (1, acc1)):
                rows = po.tile([P, P], F32, tag="rows")
                nc.scalar.activation(out=rows[:], in_=acc[:], func=AF.Copy)
                while pd["i"] % NQ != 0:
                    dummy_gather()  # scatters run on queue 0: align lane
                si = nc.gpsimd.indirect_dma_start(
                    out=pooled_nm[:, :],
                    out_offset=IndirectOffsetOnAxis(
                        ap=gcols[:, half:half + 1], axis=0),
                    in_=rows[:], in_offset=None)
                chain_pool_dma(si)

            nc.gpsimd.collective_compute(
                "AllReduce", OP.add, replica_groups=[cores],
                ins=[pooled_nm[:, :]], outs=[pooled_sum[:, :]])

            for gw in range(GW):
                ot = po.tile([P, H], F32, tag="ot")
                nc.sync.dma_start(out=ot[:],
                                  in_=pooled_sum[gw * P:(gw + 1) * P, :])
                os = po.tile([P, H], F32, tag="os")
                nc.vector.tensor_scalar(
                    out=os[:], in0=ot[:], scalar1=recip_sb[:, gw:gw + 1],
                    scalar2=None, op0=OP.mult)
                nc.sync.dma_start(out=out_d[gw * P:(gw + 1) * P, :],
                                  in_=os[:])

    return nc


# --------------------------------------------------------------------------
# entry point: full inputs -> full output
# --------------------------------------------------------------------------

_CACHE = {}


def _get_compiled(cfg, st_key, st):
    if st_key not in _CACHE:
        nc = build_nc(cfg, st)
        nc.finalize()
        _CACHE[st_key] = nc
    return _CACHE[st_key]


def kernel(x, edge_index, batch, emb_table, Ws, bs):
    cfg = Cfg()  # full problem size, hardcoded
    st, in_maps = preprocess(x, edge_index, batch, emb_table, Ws, bs, cfg)
    st_key = tuple(tuple(r) for r in st.t_cw)
    nc = _get_compiled(cfg, st_key, st)

    from concourse.bass_utils import run_bass_kernel_spmd

    res = run_bass_kernel_spmd(nc, in_maps, list(range(cfg.C)))
    return np.ascontiguousarray(res.results[0]["out"])
